# revision 1
# baseline (speedup 1.0000x reference)
"""Trainium2 Bass kernel for GQA attention (B=4, S=1024, DIM=2048, 32 Q heads /
8 KV heads, head_dim 64, rotary + causal mask, QKV + output projections).

Sharding: 8 cores = batch (4) x head-half (2). Each core computes one batch's
attention for 16 Q heads / 4 KV heads plus the partial output projection over
its 1024 y-features; the host sums the two partials per batch.

Layout: feature-major (q^T/k^T/y^T), scores computed transposed (P^T[k, q]) so
softmax sums come from a ones-column in the AV stationary. Matmuls run in
float32r (fp32 storage, 11-bit mantissa) except AV which runs bf16 (P and v).
Causal structure is data-driven from the mask input (fully-masked 128-blocks
are skipped, mixed blocks get a multiplicative mask tile).
"""

import hashlib
import sys

import numpy as np

for _p in ("/root/.axon_site/_ro/trn_rl_repo", "/opt/trn_rl_repo"):
    if _p not in sys.path:
        sys.path.append(_p)

import ml_dtypes
import concourse.bacc as bacc
import concourse.mybir as mybir
from concourse.tile import TileContext
from concourse.bass_utils import run_bass_kernel_spmd

F32 = mybir.dt.float32
F32R = mybir.dt.float32r
BF16 = mybir.dt.bfloat16
AF = mybir.ActivationFunctionType
OP = mybir.AluOpType

B, S, DIM = 4, 1024, 2048
NH, NKV, HD = 32, 8, 64
NQL, NKVL = 16, 4
N_CORES = 8
KT = S // 128
QPAIRS = 8
ND = DIM // 128
SCALE = 1.0 / 8.0


def _pairing(t):
    return (t, t + 4) if t < 4 else (t + 4, t + 8)


def _analyze_mask(M):
    """Block-classify the [S, S] bool mask (M[q, k]).

    Returns:
      runs[ki]  : list of (qs, qe, [(qt, mask_idx)]) maximal valid runs over q
      span[ki]  : (lo, hi) overall valid q range or None
      mixed     : list of unique mixed-block tiles in P^T layout [k, q]
    """
    runs, span = {}, {}
    mixed, midx = [], {}
    for ki in range(KT):
        rr, cur = [], None
        lo = hi = None
        for qt in range(KT):
            blk = M[128 * qt:128 * qt + 128, 128 * ki:128 * ki + 128]
            if (~blk).all():
                if cur is not None:
                    rr.append(tuple(cur))
                    cur = None
                continue
            mix = []
            if not blk.all():
                key = blk.tobytes()
                if key not in midx:
                    mixed.append(np.ascontiguousarray(blk.T).astype(np.float32))
                    midx[key] = len(mixed) - 1
                mix = [(qt, midx[key])]
            if cur is None:
                cur = [128 * qt, 128 * qt + 128, mix]
            else:
                cur[1] = 128 * qt + 128
                cur[2] += mix
            lo = 128 * qt if lo is None else lo
            hi = 128 * qt + 128
        if cur is not None:
            rr.append(tuple(cur))
        runs[ki] = rr
        span[ki] = (lo, hi) if lo is not None else None
    return runs, span, mixed


def _bank_subruns(runs_ki):
    """Split runs at 512 boundaries -> [(qs, qe, qc)], each inside one bank."""
    out = []
    for (qs, qe, _mix) in runs_ki:
        for qc in range(2):
            a, b = max(qs, 512 * qc), min(qe, 512 * qc + 512)
            if a < b:
                out.append((a, b, qc))
    return out


def _build_program(runs, span, n_mixed):
    nc = bacc.Bacc("TRN2", target_bir_lowering=False, debug=False,
                   num_devices=N_CORES)

    xT = nc.dram_tensor("xT", [DIM, S], F32R, kind="ExternalInput")
    wqk = nc.dram_tensor("wqk", [10, 128, ND, 128], F32R, kind="ExternalInput")
    wv = nc.dram_tensor("wv", [ND, 128, NKVL * HD], F32R, kind="ExternalInput")
    wo_t = nc.dram_tensor("wo_t", [4, 128, 8, 512], F32R, kind="ExternalInput")
    cos_d = nc.dram_tensor("cos_d", [128, S], F32, kind="ExternalInput")
    sin_d = nc.dram_tensor("sin_d", [128, S], F32, kind="ExternalInput")
    nmx = max(n_mixed, 1)
    msk_d = nc.dram_tensor("msk_d", [nmx, 128, 128], BF16, kind="ExternalInput")
    out_d = nc.dram_tensor("out", [KT, 128, DIM], F32, kind="ExternalOutput")

    with TileContext(nc) as tc:
      with tc.tile_pool(name="res", bufs=1) as res:
        qk_t = [res.tile([128, S], F32R, name=f"qk{t}", tag=f"qk{t}")
                for t in range(10)]
        v_t = [res.tile([128, NKVL * 65], BF16, name=f"v{k}", tag=f"v{k}")
               for k in range(KT)]
        y_t = [res.tile([128, S], F32R, name=f"y{t}", tag=f"y{t}")
               for t in range(QPAIRS)]
        cos_sb = res.tile([128, S], F32, name="cos_sb")
        sin_sb = res.tile([128, S], F32, name="sin_sb")
        nc.sync.dma_start(cos_sb[:], cos_d[:])
        nc.sync.dma_start(sin_sb[:], sin_d[:])
        msk_sb = [res.tile([128, 128], BF16, name=f"msk{i}", tag=f"msk{i}")
                  for i in range(n_mixed)]
        for i in range(n_mixed):
            nc.sync.dma_start(msk_sb[i][:], msk_d[i])
        ones4 = res.tile([128, NKVL], F32, name="ones4")
        nc.gpsimd.memset(ones4[:], 1.0)
        woeh = [res.tile([128, 4, 512], F32R, name=f"woeh{i}", tag="woeh",
                         bufs=2) for i in range(8)]
        osb_t = [res.tile([128, 512], F32, name=f"osb{i}", tag="osb", bufs=2)
                 for i in range(32)]

        def qkproj(st, pp, half, f, xd, dest):
            """Project feature tile f for one token half + rope into dest."""
            tsl = slice(512 * half, 512 * half + 512)
            wf = st.tile([128, ND, 128], F32R, name=f"wf{half}_{f}", tag="wf",
                         bufs=2)
            nc.sync.dma_start(wf[:], wqk[f])
            ps = pp.tile([128, 512], F32, name=f"psqk{half}_{f}", tag="psproj",
                         bufs=2)
            for d in range(ND):
                nc.tensor.matmul(ps[:], wf[:, d, :], xd[d][:],
                                 start=(d == 0), stop=(d == ND - 1))
            c0 = st.tile([128, 512], F32, name=f"c0_{half}_{f}", tag="c0", bufs=2)
            nc.scalar.copy(c0[:], ps[:])
            sw = st.tile([128, 512], F32, name=f"sw_{half}_{f}", tag="sw", bufs=2)
            for blk in range(4):
                sb = (blk ^ 1) * 32
                nc.sync.dma_start(sw[32 * blk:32 * blk + 32, :],
                                  c0[sb:sb + 32, :])
            t1 = st.tile([128, 512], F32, name=f"t1_{half}_{f}", tag="t1", bufs=1)
            t2 = st.tile([128, 512], F32, name=f"t2_{half}_{f}", tag="t2", bufs=1)
            nc.vector.tensor_mul(t1[:], c0[:], cos_sb[:, tsl])
            nc.vector.tensor_mul(t2[:], sw[:], sin_sb[:, tsl])
            nc.vector.tensor_add(dest[:, tsl], t1[:], t2[:])

        # ------------- projections + attention (one pool scope) -------
        with (
            tc.tile_pool(name="p2", bufs=1) as st,
            tc.tile_pool(name="pp2", bufs=1, space="PSUM") as pp,
        ):
            with nc.named_scope("proj"):
                for half in range(2):
                    xd = [st.tile([128, 512], F32R, name=f"x2_{half}_{d}",
                                  tag="xd2", bufs=16) for d in range(ND)]
                    for d in range(ND):
                        nc.sync.dma_start(
                            xd[d][:],
                            xT[128 * d:128 * d + 128,
                               512 * half:512 * half + 512])
                    wvt = st.tile([128, ND, NKVL * HD], F32R,
                                  name=f"wvt{half}", tag="wvt", bufs=1)
                    nc.sync.dma_start(wvt[:], wv[:].rearrange("d p c -> p d c"))
                    for tq in range(4):
                        ki = 4 * half + tq
                        psv = pp.tile([128, NKVL * HD], F32, name=f"psv{ki}",
                                      tag="psproj", bufs=2)
                        for d in range(ND):
                            nc.tensor.matmul(
                                psv[:], xd[d][:, 128 * tq:128 * tq + 128],
                                wvt[:, d, :], start=(d == 0),
                                stop=(d == ND - 1))
                        vv = v_t[ki][:].rearrange("p (u c) -> p u c",
                                                  u=NKVL, c=65)
                        nc.scalar.copy(
                            vv[:, :, 0:64],
                            psv[:].rearrange("p (u c) -> p u c", u=NKVL, c=HD))
                        nc.scalar.copy(
                            vv[:, :, 64:65],
                            ones4[:].rearrange("p (u o) -> p u o",
                                               u=NKVL, o=1))
                    for f in [8, 9] + list(range(QPAIRS)):
                        qkproj(st, pp, half, f, xd, qk_t[f])

            with nc.named_scope("attn"):
                for p in range(QPAIRS):
                    tk = 0 if p < 4 else 1
                    heads = _pairing(p)
                    ptiles = {}
                    poff = {}
                    for ki in range(KT):
                        if span[ki] is None:
                            continue
                        lo, hi = span[ki]
                        w = hi - lo
                        kwin = slice(128 * ki, 128 * ki + 128)
                        merge = w <= 512
                        if merge:
                            # both heads share one PSUM tile (side s at
                            # psum col 512s) and one exp into a packed P tile
                            psS = pp.tile([128, 1024], F32,
                                          name=f"psS{p}{ki}", tag="psS", bufs=2)
                            pt = st.tile([128, 2 * w], BF16, name=f"P{p}_{ki}",
                                         tag=f"Pm_{ki}", bufs=1)
                            for s in range(2):
                                ptiles[(s, ki)] = pt
                                poff[(s, ki)] = s * w - lo
                                bank_first = True
                                for (qs, qe, qc) in _bank_subruns(runs[ki]):
                                    nc.tensor.matmul(
                                        psS[:, qs - lo + 512 * s:
                                            qe - lo + 512 * s],
                                        qk_t[8 + tk][64 * s:64 * s + 64, kwin],
                                        qk_t[p][64 * s:64 * s + 64, qs:qe],
                                        start=bank_first, stop=True,
                                        skip_group_check=True)
                                    bank_first = False
                            psv2 = psS[:].rearrange("p (b c) -> p b c",
                                                    b=2, c=512)[:, :, 0:w]
                            ptv = pt[:].rearrange("p (b c) -> p b c",
                                                  b=2, c=w)
                            nc.scalar.activation(ptv, psv2, AF.Exp,
                                                 scale=SCALE)
                            for (qs, qe, mix) in runs[ki]:
                                for (qt, mi) in mix:
                                    for s in range(2):
                                        o = s * w + 128 * qt - lo
                                        nc.vector.tensor_mul(
                                            pt[:, o:o + 128],
                                            pt[:, o:o + 128], msk_sb[mi][:])
                        else:
                            for s in range(2):
                                psS = pp.tile([128, 1024], F32,
                                              name=f"psS{p}{ki}{s}",
                                              tag="psS", bufs=2)
                                pt = st.tile([128, w], BF16,
                                             name=f"P{p}_{s}_{ki}",
                                             tag=f"P{s}_{ki}", bufs=1)
                                ptiles[(s, ki)] = pt
                                poff[(s, ki)] = -lo
                                bank_first = {}
                                for (qs, qe, qc) in _bank_subruns(runs[ki]):
                                    st_flag = bank_first.setdefault(qc, True)
                                    bank_first[qc] = False
                                    nc.tensor.matmul(
                                        psS[:, qs:qe],
                                        qk_t[8 + tk][64 * s:64 * s + 64, kwin],
                                        qk_t[p][64 * s:64 * s + 64, qs:qe],
                                        start=st_flag, stop=True,
                                        skip_group_check=True)
                                nc.scalar.activation(pt[:], psS[:, lo:hi],
                                                     AF.Exp, scale=SCALE)
                                for (qs, qe, mix) in runs[ki]:
                                    for (qt, mi) in mix:
                                        o = 128 * qt - lo
                                        nc.vector.tensor_mul(
                                            pt[:, o:o + 128],
                                            pt[:, o:o + 128], msk_sb[mi][:])
                    for s in range(2):
                        u = heads[s] // 4
                        for qc in range(2):
                            subs = []
                            for ki in range(KT):
                                if span[ki] is None:
                                    continue
                                for (qs, qe, qq) in _bank_subruns(runs[ki]):
                                    if qq == qc:
                                        subs.append((ki, qs, qe))
                            if not subs:
                                continue
                            psyf = pp.tile([128, 512], F32,
                                           name=f"psy{p}{s}{qc}", tag="psy",
                                           bufs=2)
                            psy = psyf[0:65, :]
                            for n, (ki, qs, qe) in enumerate(subs):
                                off = poff[(s, ki)]
                                nc.tensor.matmul(
                                    psy[:, qs - 512 * qc:qe - 512 * qc],
                                    v_t[ki][:, 65 * u:65 * u + 65],
                                    ptiles[(s, ki)][:, qs + off:qe + off],
                                    start=(n == 0), stop=(n == len(subs) - 1),
                                    skip_group_check=True)
                            s_sb = st.tile([1, 512], F32, name=f"s{p}{s}{qc}",
                                           tag="srf", bufs=2)
                            nc.vector.tensor_copy(s_sb[:], psy[64:65, :])
                            rf = st.tile([1, 512], F32, name=f"rf{p}{s}{qc}",
                                         tag="srf", bufs=2)
                            nc.vector.reciprocal_approx_fast(rf[:], s_sb[:])
                            rb = st.tile([64, 512], F32, name=f"rb{p}{s}{qc}",
                                         tag="rb", bufs=2)
                            nc.gpsimd.partition_broadcast(rb[:], rf[:])
                            nc.vector.tensor_mul(
                                y_t[p][64 * s:64 * s + 64,
                                       512 * qc:512 * qc + 512],
                                psy[0:64, :], rb[:])

        # ---------------- output projection ----------------
        with (
            nc.named_scope("wo"),
            tc.tile_pool(name="pp3", bufs=1, space="PSUM") as pp,
        ):
            for i in range(8):
                ec, fh = i // 2, i % 2
                nc.sync.dma_start(woeh[i][:], wo_t[ec, :, 4 * fh:4 * fh + 4, :])
            for ec in range(4):
                psos = {}
                for fh in range(2):
                    for tt in range(KT):
                        if fh == 0:
                            psos[tt] = pp.tile([128, 512], F32,
                                               name=f"pso{ec}{tt}", tag="pso",
                                               bufs=8)
                        pso = psos[tt]
                        for f in range(4 * fh, 4 * fh + 4):
                            nc.tensor.matmul(
                                pso[:], y_t[f][:, 128 * tt:128 * tt + 128],
                                woeh[2 * ec + fh][:, f % 4, :],
                                start=(f == 0), stop=(f == 7))
                        if fh == 1:
                            osb = osb_t[8 * ec + tt % 8]
                            nc.scalar.copy(osb[:], pso[:])
                            nc.sync.dma_start(
                                out_d[tt, :, 512 * ec:512 * ec + 512], osb[:])

    nc.compile()
    return nc


_CACHE = {}


def _get_program(mask):
    M = np.asarray(mask).reshape(S, S).astype(bool)
    key = hashlib.md5(M.tobytes()).hexdigest()
    if key not in _CACHE:
        runs, span, mixed = _analyze_mask(M)
        nc = _build_program(runs, span, len(mixed))
        _CACHE[key] = (nc, mixed)
    return _CACHE[key]


def _round_fp32r(a):
    """Round fp32 -> fp32r encoding (11-bit mantissa, low 12 bits zero)."""
    b = np.ascontiguousarray(a, dtype=np.float32).view(np.uint32)
    lsb = (b >> np.uint32(12)) & np.uint32(1)
    r = (b + np.uint32(0x7FF) + lsb) & np.uint32(0xFFFFF000)
    return r.view(np.float32)


def _host_inputs(x, freqs_cis, wqkv, wo, mixed):
    """Build the 8 per-core input maps."""
    x = np.asarray(x, dtype=np.float32)
    fc = np.asarray(freqs_cis, dtype=np.float32)
    wqkv = np.asarray(wqkv, dtype=np.float32)
    wo = np.asarray(wo, dtype=np.float32)

    cosv = fc[:, :, 0].T
    sinv = fc[:, :, 1].T
    cos_t = np.ascontiguousarray(np.tile(cosv, (4, 1)))
    sin_t = np.tile(sinv, (4, 1))
    sgn = np.ones((128, 1), np.float32)
    sgn[np.arange(128) % 64 < 32] = -1.0
    sin_t = np.ascontiguousarray(sin_t * sgn)

    nmx = max(len(mixed), 1)
    msk_arr = np.zeros((nmx, 128, 128), ml_dtypes.bfloat16)
    for i, m in enumerate(mixed):
        msk_arr[i] = m.astype(ml_dtypes.bfloat16)

    j = np.arange(HD)
    refdim = 2 * (j % 32) + (j // 32)

    in_maps = []
    for b in range(B):
        xTb = np.ascontiguousarray(x[b].T)
        for h in range(2):
            rows = np.empty(1280, np.int64)
            for t in range(8):
                a, bb = _pairing(t)
                for sde, ql in enumerate((a, bb)):
                    g = h * NQL + ql
                    rows[128 * t + 64 * sde + j] = g * HD + refdim
            for tkk in range(2):
                for sde in range(2):
                    u = tkk * 2 + sde
                    g = h * NKVL + u
                    rows[1024 + 128 * tkk + 64 * sde + j] = \
                        NH * HD + g * HD + refdim
            W4 = wqkv[rows]
            wqk_a = np.ascontiguousarray(
                W4.reshape(10, 128, ND, 128).transpose(0, 3, 2, 1))
            vrows = (NH + NKV) * HD + (h * NKVL * HD) + np.arange(NKVL * HD)
            Wv = wqkv[vrows]
            wv_a = np.ascontiguousarray(
                Wv.reshape(NKVL * HD, ND, 128).transpose(1, 2, 0))
            worow = np.empty(1024, np.int64)
            dd = np.arange(HD)
            for t in range(8):
                a, bb = _pairing(t)
                for sde, ql in enumerate((a, bb)):
                    worow[128 * t + 64 * sde + dd] = (h * NQL + ql) * HD + dd
            woT = np.ascontiguousarray(wo[:, worow].T)
            wo_a = np.ascontiguousarray(
                woT.reshape(8, 128, 4, 512).transpose(2, 1, 0, 3))
            in_maps.append({
                "xT": _round_fp32r(xTb),
                "wqk": _round_fp32r(wqk_a),
                "wv": _round_fp32r(wv_a),
                "wo_t": _round_fp32r(wo_a),
                "cos_d": cos_t,
                "sin_d": sin_t,
                "msk_d": msk_arr,
            })
    return in_maps


def _run(x, freqs_cis, mask, wqkv, wo, trace=False):
    nc, mixed = _get_program(mask)
    in_maps = _host_inputs(x, freqs_cis, wqkv, wo, mixed)
    res = run_bass_kernel_spmd(nc, in_maps, list(range(N_CORES)), trace=trace)
    outs = [res.results[i]["out"].reshape(S, DIM) for i in range(N_CORES)]
    full = np.stack([outs[2 * b] + outs[2 * b + 1] for b in range(B)])
    return full.astype(np.float32), res


def kernel(x, freqs_cis, mask, wqkv, wo):
    full, _ = _run(x, freqs_cis, mask, wqkv, wo, trace=False)
    return full



# revision 10
# speedup vs baseline: 1.1336x; 1.1336x over previous
"""Trainium2 Bass kernel for GQA attention (B=4, S=1024, DIM=2048, 32 Q heads /
8 KV heads, head_dim 64, rotary + causal mask, QKV + output projections).

Sharding: 8 cores = batch (4) x head-half (2). Each core computes one batch's
attention for 16 Q heads / 4 KV heads plus the partial output projection over
its 1024 y-features; the host sums the two partials per batch.

v2: all matmul operands in bf16 (halves HBM traffic, avoids fp32r narrow-ap
penalty), fp16 output partials, and a PE-gapless schedule: scores for pair p
are interleaved right after the projection of q-tile p+1, AV/normalize are
pipelined behind the Act-engine exps, and the wo projection runs in f-outer
waves sharing one PSUM pool (no pool-close barrier).
"""

import hashlib
import sys

import numpy as np

for _p in ("/root/.axon_site/_ro/trn_rl_repo", "/opt/trn_rl_repo"):
    if _p not in sys.path:
        sys.path.append(_p)

import ml_dtypes
import concourse.bacc as bacc
import concourse.mybir as mybir
from concourse.tile import TileContext
from concourse.bass_utils import run_bass_kernel_spmd

F32 = mybir.dt.float32
BF16 = mybir.dt.bfloat16
F16 = mybir.dt.float16
AF = mybir.ActivationFunctionType

B, S, DIM = 4, 1024, 2048
NH, NKV, HD = 32, 8, 64
NQL, NKVL = 16, 4
N_CORES = 8
KT = S // 128
QPAIRS = 8
ND = DIM // 128
SCALE = 1.0 / 8.0


def _pairing(t):
    return (t, t + 4) if t < 4 else (t + 4, t + 8)


def _analyze_mask(M):
    """Block-classify the [S, S] bool mask (M[q, k]).

    Returns:
      runs[ki]  : list of (qs, qe, [(qt, mask_idx)]) maximal valid runs over q
      span[ki]  : (lo, hi) overall valid q range or None
      mixed     : list of unique mixed-block tiles in P^T layout [k, q]
    """
    runs, span = {}, {}
    mixed, midx = [], {}
    for ki in range(KT):
        rr, cur = [], None
        lo = hi = None
        for qt in range(KT):
            blk = M[128 * qt:128 * qt + 128, 128 * ki:128 * ki + 128]
            if (~blk).all():
                if cur is not None:
                    rr.append(tuple(cur))
                    cur = None
                continue
            mix = []
            if not blk.all():
                key = blk.tobytes()
                if key not in midx:
                    mixed.append(np.ascontiguousarray(blk.T).astype(np.float32))
                    midx[key] = len(mixed) - 1
                mix = [(qt, midx[key])]
            if cur is None:
                cur = [128 * qt, 128 * qt + 128, mix]
            else:
                cur[1] = 128 * qt + 128
                cur[2] += mix
            lo = 128 * qt if lo is None else lo
            hi = 128 * qt + 128
        if cur is not None:
            rr.append(tuple(cur))
        runs[ki] = rr
        span[ki] = (lo, hi) if lo is not None else None
    return runs, span, mixed


def _bank_subruns(runs_ki):
    """Split runs at 512 boundaries -> [(qs, qe, qc)], each inside one bank."""
    out = []
    for (qs, qe, _mix) in runs_ki:
        for qc in range(2):
            a, b = max(qs, 512 * qc), min(qe, 512 * qc + 512)
            if a < b:
                out.append((a, b, qc))
    return out


def _build_program(runs, span, n_mixed):
    nc = bacc.Bacc("TRN2", target_bir_lowering=False, debug=False,
                   num_devices=N_CORES)

    xT = nc.dram_tensor("xT", [DIM, S], BF16, kind="ExternalInput")
    wqk = nc.dram_tensor("wqk", [10, 128, ND, 128], BF16, kind="ExternalInput")
    wv = nc.dram_tensor("wv", [ND, 128, NKVL * HD], BF16, kind="ExternalInput")
    wo_t = nc.dram_tensor("wo_t", [4, 128, 8, 512], BF16, kind="ExternalInput")
    cos_d = nc.dram_tensor("cos_d", [128, S], BF16, kind="ExternalInput")
    sin_d = nc.dram_tensor("sin_d", [128, S], BF16, kind="ExternalInput")
    nmx = max(n_mixed, 1)
    msk_d = nc.dram_tensor("msk_d", [nmx, 128, 128], BF16, kind="ExternalInput")
    out_d = nc.dram_tensor("out", [KT, 128, DIM], F16, kind="ExternalOutput")

    with TileContext(nc) as tc:
      with tc.tile_pool(name="res", bufs=1) as res:
        qk_t = [res.tile([128, S], BF16, name=f"qk{t}", tag=f"qk{t}")
                for t in range(10)]
        v_t = [res.tile([128, NKVL * 65], BF16, name=f"v{k}", tag=f"v{k}")
               for k in range(KT)]
        y_t = [res.tile([128, S], BF16, name=f"y{t}", tag=f"y{t}")
               for t in range(QPAIRS)]
        cos_sb = res.tile([128, S], BF16, name="cos_sb")
        sin_sb = res.tile([128, S], BF16, name="sin_sb")
        msk_sb = [res.tile([128, 128], BF16, name=f"msk{i}", tag=f"msk{i}")
                  for i in range(n_mixed)]
        ones4 = res.tile([128, NKVL], BF16, name="ones4")
        woeh = [res.tile([128, 4, 512], BF16, name=f"woeh{i}", tag="woeh",
                         bufs=8) for i in range(8)]
        osb_t = [res.tile([128, 512], F16, name=f"osb{i}", tag="osb", bufs=8)
                 for i in range(32)]

        with (
            tc.tile_pool(name="st", bufs=1) as st,
            tc.tile_pool(name="pp", bufs=1, space="PSUM") as pp,
        ):
            # xd tiles for both halves up front (32 DMAs, first ones land in
            # <1us); weights stream per-f with bufs=3 prefetch.
            xd = {}
            for half in range(2):
                for d in range(ND):
                    xd[(half, d)] = st.tile([128, 512], BF16,
                                            name=f"x_{half}_{d}", tag="xd",
                                            bufs=32)
                    nc.sync.dma_start(
                        xd[(half, d)][:],
                        xT[128 * d:128 * d + 128,
                           512 * half:512 * half + 512])
            nc.gpsimd.memset(ones4[:], 1.0)
            nc.sync.dma_start(cos_sb[:], cos_d[:])
            nc.sync.dma_start(sin_sb[:], sin_d[:])
            for i in range(n_mixed):
                nc.sync.dma_start(msk_sb[i][:], msk_d[i])
            wvt = st.tile([128, ND, NKVL * HD], BF16, name="wvt", tag="wvt",
                          bufs=1)
            nc.sync.dma_start(wvt[:], wv[:].rearrange("d p c -> p d c"))
            for i in range(8):
                ec, fh = i // 2, i % 2
                nc.sync.dma_start(woeh[i][:], wo_t[ec, :, 4 * fh:4 * fh + 4, :])

            def qkproj(half, f):
                """Project feature tile f for one token half + rope into
                qk_t[f][:, half]."""
                tsl = slice(512 * half, 512 * half + 512)
                wf = st.tile([128, ND, 128], BF16, name=f"wf{half}_{f}",
                             tag="wf", bufs=3)
                nc.sync.dma_start(wf[:], wqk[f])
                ps = pp.tile([128, 512], F32, name=f"psqk{half}_{f}",
                             tag="flex", bufs=2)
                for d in range(ND):
                    nc.tensor.matmul(ps[:], wf[:, d, :], xd[(half, d)][:],
                                     start=(d == 0), stop=(d == ND - 1))
                c0 = st.tile([128, 512], BF16, name=f"c0_{half}_{f}", tag="c0",
                             bufs=2)
                nc.scalar.copy(c0[:], ps[:])
                sw = st.tile([128, 512], BF16, name=f"sw_{half}_{f}", tag="sw",
                             bufs=2)
                for blk in range(4):
                    sb = (blk ^ 1) * 32
                    nc.sync.dma_start(sw[32 * blk:32 * blk + 32, :],
                                      c0[sb:sb + 32, :])
                t1 = st.tile([128, 512], BF16, name=f"t1_{half}_{f}", tag="t1",
                             bufs=2)
                t2 = st.tile([128, 512], BF16, name=f"t2_{half}_{f}", tag="t2",
                             bufs=2)
                nc.vector.tensor_mul(t1[:], c0[:], cos_sb[:, tsl])
                nc.vector.tensor_mul(t2[:], sw[:], sin_sb[:, tsl])
                nc.vector.tensor_add(qk_t[f][:, tsl], t1[:], t2[:])

            def vproj(half):
                for tq in range(4):
                    ki = 4 * half + tq
                    psv = pp.tile([128, NKVL * HD], F32, name=f"psv{ki}",
                                  tag="flex", bufs=2)
                    for d in range(ND):
                        nc.tensor.matmul(
                            psv[:], xd[(half, d)][:, 128 * tq:128 * tq + 128],
                            wvt[:, d, :], start=(d == 0),
                            stop=(d == ND - 1))
                    vv = v_t[ki][:].rearrange("p (u c) -> p u c", u=NKVL, c=65)
                    nc.scalar.copy(
                        vv[:, :, 0:64],
                        psv[:].rearrange("p (u c) -> p u c", u=NKVL, c=HD))
                    nc.scalar.copy(
                        vv[:, :, 64:65],
                        ones4[:].rearrange("p (u o) -> p u o", u=NKVL, o=1))

            def scores(p):
                """QK^T + exp for pair p; returns (ptiles, poff)."""
                tk = 0 if p < 4 else 1
                ptiles, poff = {}, {}
                for ki in range(KT):
                    if span[ki] is None:
                        continue
                    lo, hi = span[ki]
                    w = hi - lo
                    kwin = slice(128 * ki, 128 * ki + 128)
                    merge = w <= 512
                    if merge:
                        psS = pp.tile([128, 1024], F32, name=f"psS{p}{ki}",
                                      tag="psS", bufs=3)
                        pt = st.tile([128, 2 * w], BF16, name=f"P{p}_{ki}",
                                     tag=f"Pm_{ki}_{p % 2}", bufs=1)
                        for s in range(2):
                            ptiles[(s, ki)] = pt
                            poff[(s, ki)] = s * w - lo
                            bank_first = True
                            for (qs, qe, qc) in _bank_subruns(runs[ki]):
                                nc.tensor.matmul(
                                    psS[:, qs - lo + 512 * s:
                                        qe - lo + 512 * s],
                                    qk_t[8 + tk][64 * s:64 * s + 64, kwin],
                                    qk_t[p][64 * s:64 * s + 64, qs:qe],
                                    start=bank_first, stop=True,
                                    skip_group_check=True)
                                bank_first = False
                        psv2 = psS[:].rearrange("p (b c) -> p b c",
                                                b=2, c=512)[:, :, 0:w]
                        ptv = pt[:].rearrange("p (b c) -> p b c", b=2, c=w)
                        nc.scalar.activation(ptv, psv2, AF.Exp, scale=SCALE)
                        for (qs, qe, mix) in runs[ki]:
                            for (qt, mi) in mix:
                                for s in range(2):
                                    o = s * w + 128 * qt - lo
                                    nc.vector.tensor_mul(
                                        pt[:, o:o + 128],
                                        pt[:, o:o + 128], msk_sb[mi][:])
                    else:
                        for s in range(2):
                            psS = pp.tile([128, 1024], F32,
                                          name=f"psS{p}{ki}{s}",
                                          tag="psS", bufs=3)
                            pt = st.tile([128, w], BF16, name=f"P{p}_{s}_{ki}",
                                         tag=f"P{s}_{ki}_{p % 2}", bufs=1)
                            ptiles[(s, ki)] = pt
                            poff[(s, ki)] = -lo
                            bank_first = {}
                            for (qs, qe, qc) in _bank_subruns(runs[ki]):
                                st_flag = bank_first.setdefault(qc, True)
                                bank_first[qc] = False
                                nc.tensor.matmul(
                                    psS[:, qs:qe],
                                    qk_t[8 + tk][64 * s:64 * s + 64, kwin],
                                    qk_t[p][64 * s:64 * s + 64, qs:qe],
                                    start=st_flag, stop=True,
                                    skip_group_check=True)
                            nc.scalar.activation(pt[:], psS[:, lo:hi],
                                                 AF.Exp, scale=SCALE)
                            for (qs, qe, mix) in runs[ki]:
                                for (qt, mi) in mix:
                                    o = 128 * qt - lo
                                    nc.vector.tensor_mul(
                                        pt[:, o:o + 128],
                                        pt[:, o:o + 128], msk_sb[mi][:])
                return ptiles, poff

            def av(p, ptiles, poff):
                """AV + softmax-normalize into y_t[p] for pair p."""
                heads = _pairing(p)
                for s in range(2):
                    u = heads[s] // 4
                    for qc in range(2):
                        subs = []
                        for ki in range(KT):
                            if span[ki] is None:
                                continue
                            for (qs, qe, qq) in _bank_subruns(runs[ki]):
                                if qq == qc:
                                    subs.append((ki, qs, qe))
                        if not subs:
                            continue
                        psyf = pp.tile([128, 512], F32, name=f"psy{p}{s}{qc}",
                                       tag="flex", bufs=2)
                        psy = psyf[0:65, :]
                        for n, (ki, qs, qe) in enumerate(subs):
                            off = poff[(s, ki)]
                            nc.tensor.matmul(
                                psy[:, qs - 512 * qc:qe - 512 * qc],
                                v_t[ki][:, 65 * u:65 * u + 65],
                                ptiles[(s, ki)][:, qs + off:qe + off],
                                start=(n == 0), stop=(n == len(subs) - 1),
                                skip_group_check=True)
                        s_sb = st.tile([1, 512], F32, name=f"s{p}{s}{qc}",
                                       tag="ssb", bufs=2)
                        nc.vector.tensor_copy(s_sb[:], psy[64:65, :])
                        rf = st.tile([1, 512], F32, name=f"rf{p}{s}{qc}",
                                     tag="srf", bufs=2)
                        nc.vector.reciprocal_approx_fast(rf[:], s_sb[:])
                        rb = st.tile([64, 512], F32, name=f"rb{p}{s}{qc}",
                                     tag="rb", bufs=2)
                        nc.gpsimd.partition_broadcast(rb[:], rf[:])
                        nc.vector.tensor_mul(
                            y_t[p][64 * s:64 * s + 64,
                                   512 * qc:512 * qc + 512],
                            psy[0:64, :], rb[:])

            # ---- PE-ordered schedule ----
            with nc.named_scope("proj0"):
                for f in [8, 9] + list(range(QPAIRS)):
                    qkproj(0, f)
                vproj(0)
            with nc.named_scope("proj1"):
                qkproj(1, 8)
                qkproj(1, 9)
                vproj(1)
                qkproj(1, 0)
                qkproj(1, 1)
            # Interleave: proj(f=p+1) | scores(p) | AV(p-1) so the Act engine's
            # exp stream (the attn-phase pacer) stays just ahead of AV while
            # the PE never idles; P tiles are double-buffered by pair parity.
            ptp = {}
            with nc.named_scope("attn"):
                ptp[0] = scores(0)
                for p in range(1, 7):
                    qkproj(1, p + 1)
                    ptp[p] = scores(p)
                    av(p - 1, *ptp[p - 1])
                ptp[7] = scores(7)
                av(6, *ptp[6])
                av(7, *ptp[7])

            # ---- output projection: f-outer waves of 2 psum groups ----
            with nc.named_scope("wo"):
                for wave in range(16):
                    ec, th = wave // 4, wave % 4
                    tts = (2 * th, 2 * th + 1)
                    psos = {}
                    for tt in tts:
                        psos[tt] = pp.tile([128, 512], F32,
                                           name=f"pso{ec}{tt}", tag="flex",
                                           bufs=2)
                    for f in range(QPAIRS):
                        wt = woeh[2 * ec + f // 4]
                        for tt in tts:
                            nc.tensor.matmul(
                                psos[tt][:],
                                y_t[f][:, 128 * tt:128 * tt + 128],
                                wt[:, f % 4, :],
                                start=(f == 0), stop=(f == 7))
                    for tt in tts:
                        osb = osb_t[8 * ec + tt]
                        nc.scalar.copy(osb[:], psos[tt][:])
                        nc.sync.dma_start(
                            out_d[tt, :, 512 * ec:512 * ec + 512], osb[:])

    nc.compile()
    return nc


_CACHE = {}


def _get_program(mask):
    M = np.asarray(mask).reshape(S, S).astype(bool)
    key = hashlib.md5(M.tobytes()).hexdigest()
    if key not in _CACHE:
        runs, span, mixed = _analyze_mask(M)
        nc = _build_program(runs, span, len(mixed))
        _CACHE[key] = (nc, mixed)
    return _CACHE[key]


def _host_inputs(x, freqs_cis, wqkv, wo, mixed):
    """Build the 8 per-core input maps."""
    bf = ml_dtypes.bfloat16
    x = np.asarray(x, dtype=np.float32)
    fc = np.asarray(freqs_cis, dtype=np.float32)
    wqkv = np.asarray(wqkv, dtype=np.float32)
    wo = np.asarray(wo, dtype=np.float32)

    cosv = fc[:, :, 0].T
    sinv = fc[:, :, 1].T
    cos_t = np.ascontiguousarray(np.tile(cosv, (4, 1))).astype(bf)
    sin_t = np.tile(sinv, (4, 1))
    sgn = np.ones((128, 1), np.float32)
    sgn[np.arange(128) % 64 < 32] = -1.0
    sin_t = np.ascontiguousarray(sin_t * sgn).astype(bf)

    nmx = max(len(mixed), 1)
    msk_arr = np.zeros((nmx, 128, 128), bf)
    for i, m in enumerate(mixed):
        msk_arr[i] = m.astype(bf)

    j = np.arange(HD)
    refdim = 2 * (j % 32) + (j // 32)

    in_maps = []
    for b in range(B):
        xTb = np.ascontiguousarray(x[b].T).astype(bf)
        for h in range(2):
            rows = np.empty(1280, np.int64)
            for t in range(8):
                a, bb = _pairing(t)
                for sde, ql in enumerate((a, bb)):
                    g = h * NQL + ql
                    rows[128 * t + 64 * sde + j] = g * HD + refdim
            for tkk in range(2):
                for sde in range(2):
                    u = tkk * 2 + sde
                    g = h * NKVL + u
                    rows[1024 + 128 * tkk + 64 * sde + j] = \
                        NH * HD + g * HD + refdim
            W4 = wqkv[rows]
            wqk_a = np.ascontiguousarray(
                W4.reshape(10, 128, ND, 128).transpose(0, 3, 2, 1)).astype(bf)
            vrows = (NH + NKV) * HD + (h * NKVL * HD) + np.arange(NKVL * HD)
            Wv = wqkv[vrows]
            wv_a = np.ascontiguousarray(
                Wv.reshape(NKVL * HD, ND, 128).transpose(1, 2, 0)).astype(bf)
            worow = np.empty(1024, np.int64)
            dd = np.arange(HD)
            for t in range(8):
                a, bb = _pairing(t)
                for sde, ql in enumerate((a, bb)):
                    worow[128 * t + 64 * sde + dd] = (h * NQL + ql) * HD + dd
            woT = np.ascontiguousarray(wo[:, worow].T)
            wo_a = np.ascontiguousarray(
                woT.reshape(8, 128, 4, 512).transpose(2, 1, 0, 3)).astype(bf)
            in_maps.append({
                "xT": xTb,
                "wqk": wqk_a,
                "wv": wv_a,
                "wo_t": wo_a,
                "cos_d": cos_t,
                "sin_d": sin_t,
                "msk_d": msk_arr,
            })
    return in_maps


def _run(x, freqs_cis, mask, wqkv, wo, trace=False):
    nc, mixed = _get_program(mask)
    in_maps = _host_inputs(x, freqs_cis, wqkv, wo, mixed)
    res = run_bass_kernel_spmd(nc, in_maps, list(range(N_CORES)), trace=trace)
    outs = [res.results[i]["out"].reshape(S, DIM).astype(np.float32)
            for i in range(N_CORES)]
    full = np.stack([outs[2 * b] + outs[2 * b + 1] for b in range(B)])
    return full.astype(np.float32), res


def kernel(x, freqs_cis, mask, wqkv, wo):
    full, _ = _run(x, freqs_cis, mask, wqkv, wo, trace=False)
    return full


# revision 17
# speedup vs baseline: 1.4491x; 1.2783x over previous
"""Trainium2 Bass kernel for GQA attention (B=4, S=1024, DIM=2048, 32 Q heads /
8 KV heads, head_dim 64, rotary + causal mask, QKV + output projections).

Sharding: 8 cores = batch (4) x head-half (2). Each core computes one batch's
attention for 16 Q heads / 4 KV heads plus the partial output projection over
its 1024 y-features; the host sums the partials.

v3 schedule: bf16 operands (fp32 PSUM accumulation), fp16 partial outputs.
Projection processes both token halves per weight tile (weights DMA'd once).
Per-f iterations interleave proj(f+1) | scores(f) | AV(f-2) so the Act
engine's exp stream — the attention pacer — overlaps PE work; the causal
mask is added in PSUM via an identity-stationary bias matmul (no post-exp
vector masking).  The wo projection is split into two f-halves written to
separate DRAM partials (summed on host): the first half interleaves into the
attention tail, only the second trails.
"""

import hashlib
import sys

import numpy as np

for _p in ("/root/.axon_site/_ro/trn_rl_repo", "/opt/trn_rl_repo"):
    if _p not in sys.path:
        sys.path.append(_p)

import ml_dtypes
import concourse.bacc as bacc
import concourse.mybir as mybir
from concourse.tile import TileContext
from concourse.bass_utils import run_bass_kernel_spmd

F32 = mybir.dt.float32
BF16 = mybir.dt.bfloat16
F16 = mybir.dt.float16
AF = mybir.ActivationFunctionType

B, S, DIM = 4, 1024, 2048
NH, NKV, HD = 32, 8, 64
NQL, NKVL = 16, 4
N_CORES = 8
KT = S // 128
QPAIRS = 8
ND = DIM // 128
SCALE = 1.0 / 8.0


def _pairing(t):
    return (t, t + 4) if t < 4 else (t + 4, t + 8)


def _analyze_mask(M):
    """Block-classify the [S, S] bool mask (M[q, k]).

    Returns:
      runs[ki]  : list of (qs, qe, [(qt, mask_idx)]) maximal valid runs over q
      span[ki]  : (lo, hi) overall valid q range or None
      mixed     : list of unique mixed-block tiles in P^T layout [k, q]
    """
    runs, span = {}, {}
    mixed, midx = [], {}
    for ki in range(KT):
        rr, cur = [], None
        lo = hi = None
        for qt in range(KT):
            blk = M[128 * qt:128 * qt + 128, 128 * ki:128 * ki + 128]
            if (~blk).all():
                if cur is not None:
                    rr.append(tuple(cur))
                    cur = None
                continue
            mix = []
            if not blk.all():
                key = blk.tobytes()
                if key not in midx:
                    mixed.append(np.ascontiguousarray(blk.T).astype(np.float32))
                    midx[key] = len(mixed) - 1
                mix = [(qt, midx[key])]
            if cur is None:
                cur = [128 * qt, 128 * qt + 128, mix]
            else:
                cur[1] = 128 * qt + 128
                cur[2] += mix
            lo = 128 * qt if lo is None else lo
            hi = 128 * qt + 128
        if cur is not None:
            rr.append(tuple(cur))
        runs[ki] = rr
        span[ki] = (lo, hi) if lo is not None else None
    return runs, span, mixed


def _bank_subruns(runs_ki):
    """Split runs at 512 boundaries -> [(qs, qe, qc)], each inside one bank."""
    out = []
    for (qs, qe, _mix) in runs_ki:
        for qc in range(2):
            a, b = max(qs, 512 * qc), min(qe, 512 * qc + 512)
            if a < b:
                out.append((a, b, qc))
    return out


def _build_program(runs, span, n_mixed):
    nc = bacc.Bacc("TRN2", target_bir_lowering=False, debug=False,
                   num_devices=N_CORES)

    xT = nc.dram_tensor("xT", [DIM, S], BF16, kind="ExternalInput")
    wqk = nc.dram_tensor("wqk", [10, 128, ND, 128], BF16, kind="ExternalInput")
    wv = nc.dram_tensor("wv", [ND, 128, NKVL * HD], BF16, kind="ExternalInput")
    wo_t = nc.dram_tensor("wo_t", [4, 128, 8, 512], BF16, kind="ExternalInput")
    cos_d = nc.dram_tensor("cos_d", [128, S], BF16, kind="ExternalInput")
    sin_d = nc.dram_tensor("sin_d", [128, S], BF16, kind="ExternalInput")
    eye_d = nc.dram_tensor("eye_d", [128, 128], BF16, kind="ExternalInput")
    nmx = max(n_mixed, 1)
    msk_d = nc.dram_tensor("msk_d", [nmx, 128, 128], BF16, kind="ExternalInput")
    outa_d = nc.dram_tensor("outa", [KT, 128, DIM], F16, kind="ExternalOutput")
    outb_d = nc.dram_tensor("outb", [KT, 128, DIM], F16, kind="ExternalOutput")

    with TileContext(nc) as tc:
      with tc.tile_pool(name="res", bufs=1) as res:
        qk_t = [res.tile([128, S], BF16, name=f"qk{t}", tag=f"qk{t}")
                for t in range(10)]
        v_t = [res.tile([128, NKVL * 65], BF16, name=f"v{k}", tag=f"v{k}")
               for k in range(KT)]
        y_t = [res.tile([128, S], BF16, name=f"y{t}", tag=f"y{t}")
               for t in range(QPAIRS)]
        cos_sb = res.tile([128, S], BF16, name="cos_sb")
        sin_sb = res.tile([128, S], BF16, name="sin_sb")
        eye_sb = res.tile([128, 128], BF16, name="eye_sb")
        msk_sb = [res.tile([128, 128], BF16, name=f"msk{i}", tag=f"msk{i}")
                  for i in range(n_mixed)]
        ones4 = res.tile([128, NKVL], BF16, name="ones4")
        woeh = [res.tile([128, 4, 512], BF16, name=f"woeh{i}", tag="woeh",
                         bufs=8) for i in range(8)]
        osb_t = [res.tile([128, 512], F16, name=f"osb{i}", tag="osb", bufs=6)
                 for i in range(64)]

        with (
            tc.tile_pool(name="st", bufs=1) as st,
            tc.tile_pool(name="pp", bufs=1, space="PSUM") as pp,
        ):
            # ---- DMA priority order: first-needed first ----
            wf = {}
            for f in [8, 9] + list(range(QPAIRS)):
                wf[f] = st.tile([128, ND, 128], BF16, name=f"wf{f}", tag="wf",
                                bufs=2)
            nc.sync.dma_start(wf[8][:], wqk[8])
            xd = {}
            for half in range(2):
                for d in range(ND):
                    xd[(half, d)] = st.tile([128, 512], BF16,
                                            name=f"x_{half}_{d}", tag="xd",
                                            bufs=32)
                    nc.sync.dma_start(
                        xd[(half, d)][:],
                        xT[128 * d:128 * d + 128,
                           512 * half:512 * half + 512])
            nc.sync.dma_start(cos_sb[:], cos_d[:])
            nc.sync.dma_start(sin_sb[:], sin_d[:])
            nc.sync.dma_start(eye_sb[:], eye_d[:])
            for i in range(n_mixed):
                nc.sync.dma_start(msk_sb[i][:], msk_d[i])
            nc.gpsimd.memset(ones4[:], 1.0)
            wvt = st.tile([128, ND, NKVL * HD], BF16, name="wvt", tag="wvt",
                          bufs=1)
            nc.sync.dma_start(wvt[:], wv[:].rearrange("d p c -> p d c"))

            def qkproj(f, dma=True):
                """Project feature tile f for both token halves + rope."""
                if dma:
                    nc.sync.dma_start(wf[f][:], wqk[f])
                for half in range(2):
                    tsl = slice(512 * half, 512 * half + 512)
                    ps = pp.tile([128, 512], F32, name=f"psqk{half}_{f}",
                                 tag="flex", bufs=2)
                    for d in range(ND):
                        nc.tensor.matmul(ps[:], wf[f][:, d, :],
                                         xd[(half, d)][:],
                                         start=(d == 0), stop=(d == ND - 1))
                    c0 = st.tile([128, 512], BF16, name=f"c0_{half}_{f}",
                                 tag="c0", bufs=2)
                    nc.scalar.copy(c0[:], ps[:])
                    sw = st.tile([128, 512], BF16, name=f"sw_{half}_{f}",
                                 tag="sw", bufs=2)
                    for blk in range(4):
                        sb = (blk ^ 1) * 32
                        nc.sync.dma_start(sw[32 * blk:32 * blk + 32, :],
                                          c0[sb:sb + 32, :])
                    t1 = st.tile([128, 512], BF16, name=f"t1_{half}_{f}",
                                 tag="t1", bufs=2)
                    t2 = st.tile([128, 512], BF16, name=f"t2_{half}_{f}",
                                 tag="t2", bufs=2)
                    nc.vector.tensor_mul(t1[:], c0[:], cos_sb[:, tsl])
                    nc.vector.tensor_mul(t2[:], sw[:], sin_sb[:, tsl])
                    nc.vector.tensor_add(qk_t[f][:, tsl], t1[:], t2[:])

            def vproj(half):
                for tq in range(4):
                    ki = 4 * half + tq
                    psv = pp.tile([128, NKVL * HD], F32, name=f"psv{ki}",
                                  tag="flex", bufs=2)
                    for d in range(ND):
                        nc.tensor.matmul(
                            psv[:], xd[(half, d)][:, 128 * tq:128 * tq + 128],
                            wvt[:, d, :], start=(d == 0),
                            stop=(d == ND - 1))
                    vv = v_t[ki][:].rearrange("p (u c) -> p u c", u=NKVL, c=65)
                    nc.vector.tensor_copy(
                        vv[:, :, 0:64],
                        psv[:].rearrange("p (u c) -> p u c", u=NKVL, c=HD))
                    nc.gpsimd.tensor_copy(
                        vv[:, :, 64:65],
                        ones4[:].rearrange("p (u o) -> p u o", u=NKVL, o=1))

            def scores(p):
                """QK^T (+causal bias in PSUM) + exp for pair p."""
                tk = 0 if p < 4 else 1
                ptiles, poff = {}, {}
                for ki in range(KT):
                    if span[ki] is None:
                        continue
                    lo, hi = span[ki]
                    w = hi - lo
                    kwin = slice(128 * ki, 128 * ki + 128)
                    merge = w <= 512
                    if merge:
                        psS = pp.tile([128, 1024], F32, name=f"psS{p}{ki}",
                                      tag="psS", bufs=3)
                        pt = st.tile([128, 2 * w], BF16, name=f"P{p}_{ki}",
                                     tag=f"Pm_{ki}_{p % 3}", bufs=1)
                        for s in range(2):
                            ptiles[(s, ki)] = pt
                            poff[(s, ki)] = s * w - lo
                            ops = []
                            for (qs, qe, qc) in _bank_subruns(runs[ki]):
                                ops.append((
                                    psS[:, qs - lo + 512 * s:
                                        qe - lo + 512 * s],
                                    qk_t[8 + tk][64 * s:64 * s + 64, kwin],
                                    qk_t[p][64 * s:64 * s + 64, qs:qe]))
                            for (qs, qe, mix) in runs[ki]:
                                for (qt, mi) in mix:
                                    o = 128 * qt - lo + 512 * s
                                    ops.append((psS[:, o:o + 128], eye_sb[:],
                                                msk_sb[mi][:]))
                            for n, (o_, l_, r_) in enumerate(ops):
                                nc.tensor.matmul(
                                    o_, l_, r_, start=(n == 0),
                                    stop=(n == len(ops) - 1),
                                    skip_group_check=True)
                        psv2 = psS[:].rearrange("p (b c) -> p b c",
                                                b=2, c=512)[:, :, 0:w]
                        ptv = pt[:].rearrange("p (b c) -> p b c", b=2, c=w)
                        nc.scalar.activation(ptv, psv2, AF.Exp, scale=SCALE)
                        yield
                    else:
                        for s in range(2):
                            psS = pp.tile([128, 1024], F32,
                                          name=f"psS{p}{ki}{s}",
                                          tag="psS", bufs=3)
                            pt = st.tile([128, w], BF16, name=f"P{p}_{s}_{ki}",
                                         tag=f"P{s}_{ki}_{p % 3}", bufs=1)
                            ptiles[(s, ki)] = pt
                            poff[(s, ki)] = -lo
                            bank_first = {}
                            ops = []
                            for (qs, qe, qc) in _bank_subruns(runs[ki]):
                                st_flag = bank_first.setdefault(qc, True)
                                bank_first[qc] = False
                                ops.append((st_flag, psS[:, qs:qe],
                                            qk_t[8 + tk][64 * s:64 * s + 64,
                                                         kwin],
                                            qk_t[p][64 * s:64 * s + 64,
                                                    qs:qe]))
                            for (qs, qe, mix) in runs[ki]:
                                for (qt, mi) in mix:
                                    o = 128 * qt
                                    ops.append((False, psS[:, o:o + 128],
                                                eye_sb[:], msk_sb[mi][:]))
                            for n, (sf, o_, l_, r_) in enumerate(ops):
                                nc.tensor.matmul(
                                    o_, l_, r_, start=sf,
                                    stop=(n == len(ops) - 1),
                                    skip_group_check=True)
                            nc.scalar.activation(pt[:], psS[:, lo:hi],
                                                 AF.Exp, scale=SCALE)
                            yield
                # stash for av()
                ptp[p] = (ptiles, poff)

            def av(p):
                """AV + softmax-normalize into y_t[p] for pair p."""
                ptiles, poff = ptp[p]
                heads = _pairing(p)
                for s in range(2):
                    u = heads[s] // 4
                    for qc in range(2):
                        subs = []
                        for ki in range(KT):
                            if span[ki] is None:
                                continue
                            for (qs, qe, qq) in _bank_subruns(runs[ki]):
                                if qq == qc:
                                    subs.append((ki, qs, qe))
                        if not subs:
                            continue
                        psyf = pp.tile([128, 512], F32, name=f"psy{p}{s}{qc}",
                                       tag="flex", bufs=2)
                        psy = psyf[0:65, :]
                        for n, (ki, qs, qe) in enumerate(subs):
                            off = poff[(s, ki)]
                            nc.tensor.matmul(
                                psy[:, qs - 512 * qc:qe - 512 * qc],
                                v_t[ki][:, 65 * u:65 * u + 65],
                                ptiles[(s, ki)][:, qs + off:qe + off],
                                start=(n == 0), stop=(n == len(subs) - 1),
                                skip_group_check=True)
                        s_sb = st.tile([1, 512], F32, name=f"s{p}{s}{qc}",
                                       tag="ssb", bufs=2)
                        nc.vector.tensor_copy(s_sb[:], psy[64:65, :])
                        rf = st.tile([1, 512], F32, name=f"rf{p}{s}{qc}",
                                     tag="srf", bufs=2)
                        nc.vector.reciprocal_approx_fast(rf[:], s_sb[:])
                        rb = st.tile([64, 512], F32, name=f"rb{p}{s}{qc}",
                                     tag="rb", bufs=2)
                        nc.gpsimd.partition_broadcast(rb[:], rf[:])
                        nc.vector.tensor_mul(
                            y_t[p][64 * s:64 * s + 64,
                                   512 * qc:512 * qc + 512],
                            psy[0:64, :], rb[:])
                        yield

            def wo_wave(wave, fh):
                """One wo wave: 2 psum groups over f-half fh (4 f's)."""
                ec, th = wave // 4, wave % 4
                tts = (2 * th, 2 * th + 1)
                psos = {}
                for tt in tts:
                    psos[tt] = pp.tile([128, 512], F32,
                                       name=f"pso{fh}{ec}{tt}", tag="flex",
                                       bufs=2)
                for f in range(4 * fh, 4 * fh + 4):
                    wt = woeh[2 * ec + f // 4]
                    for tt in tts:
                        nc.tensor.matmul(
                            psos[tt][:],
                            y_t[f][:, 128 * tt:128 * tt + 128],
                            wt[:, f % 4, :],
                            start=(f % 4 == 0), stop=(f % 4 == 3))
                out_dram = outa_d if fh == 0 else outb_d
                for tt in tts:
                    osb = osb_t[32 * fh + 8 * ec + tt]
                    if fh == 0:
                        nc.vector.tensor_copy(osb[:], psos[tt][:])
                    else:
                        nc.scalar.copy(osb[:], psos[tt][:])
                    nc.sync.dma_start(
                        out_dram[tt, :, 512 * ec:512 * ec + 512], osb[:])

            def drain(gen):
                if gen is not None:
                    for _ in gen:
                        pass

            def step(gen, n=1):
                if gen is None:
                    return
                for _ in range(n):
                    try:
                        next(gen)
                    except StopIteration:
                        break

            ptp = {}
            sgen = {}
            agen = {}
            with nc.named_scope("main"):
                qkproj(8, dma=False)
                qkproj(9)
                vproj(0)
                qkproj(0)
                vproj(1)
                qkproj(1)
                # iterations f=2..7: proj(f) | scores(f-2) | AV(f-4)
                for f in range(2, 8):
                    if f == 6:
                        # prefetch wo weights (needed from the attn tail on)
                        for i in range(8):
                            ec, fh2 = i // 2, i % 2
                            nc.sync.dma_start(
                                woeh[i][:],
                                wo_t[ec, :, 4 * fh2:4 * fh2 + 4, :])
                    p = f - 2
                    sgen[p] = scores(p)
                    qkproj(f)
                    step(sgen[p], 6)
                    agen[p - 2] = av(p - 2) if p >= 2 else None
                    step(agen.get(p - 2), 2)
                    drain(sgen[p])
                    step(agen.get(p - 2), 2)
                    drain(agen.get(p - 2))
                # tail: scores(6,7), AV(4..7), wo-A waves interleaved
                sgen[6] = scores(6)
                drain(sgen[6])
                agen[4] = av(4)
                drain(agen[4])
                sgen[7] = scores(7)
                drain(sgen[7])
                agen[5] = av(5)
                step(agen[5], 2)
                wo_wave(0, 0)
                drain(agen[5])
                wo_wave(1, 0)
                agen[6] = av(6)
                step(agen[6], 2)
                wo_wave(2, 0)
                drain(agen[6])
                wo_wave(3, 0)
                agen[7] = av(7)
                step(agen[7], 2)
                wo_wave(4, 0)
                drain(agen[7])
                for wave in range(5, 16):
                    wo_wave(wave, 0)
                for wave in range(16):
                    wo_wave(wave, 1)

    nc.compile()
    return nc


_CACHE = {}


def _get_program(mask):
    M = np.asarray(mask).reshape(S, S).astype(bool)
    key = hashlib.md5(M.tobytes()).hexdigest()
    if key not in _CACHE:
        runs, span, mixed = _analyze_mask(M)
        nc = _build_program(runs, span, len(mixed))
        _CACHE[key] = (nc, mixed)
    return _CACHE[key]


def _host_inputs(x, freqs_cis, wqkv, wo, mixed):
    """Build the 8 per-core input maps."""
    bf = ml_dtypes.bfloat16
    x = np.asarray(x, dtype=np.float32)
    fc = np.asarray(freqs_cis, dtype=np.float32)
    wqkv = np.asarray(wqkv, dtype=np.float32)
    wo = np.asarray(wo, dtype=np.float32)

    cosv = fc[:, :, 0].T
    sinv = fc[:, :, 1].T
    cos_t = np.ascontiguousarray(np.tile(cosv, (4, 1))).astype(bf)
    sin_t = np.tile(sinv, (4, 1))
    sgn = np.ones((128, 1), np.float32)
    sgn[np.arange(128) % 64 < 32] = -1.0
    sin_t = np.ascontiguousarray(sin_t * sgn).astype(bf)

    nmx = max(len(mixed), 1)
    # mask shipped as additive bias tiles: 0 where valid, -30000 where masked
    msk_arr = np.zeros((nmx, 128, 128), bf)
    for i, m in enumerate(mixed):
        msk_arr[i] = ((m - 1.0) * 30000.0).astype(bf)
    eye = np.eye(128, dtype=bf)

    j = np.arange(HD)
    refdim = 2 * (j % 32) + (j // 32)

    in_maps = []
    for b in range(B):
        xTb = np.ascontiguousarray(x[b].T).astype(bf)
        for h in range(2):
            rows = np.empty(1280, np.int64)
            for t in range(8):
                a, bb = _pairing(t)
                for sde, ql in enumerate((a, bb)):
                    g = h * NQL + ql
                    rows[128 * t + 64 * sde + j] = g * HD + refdim
            for tkk in range(2):
                for sde in range(2):
                    u = tkk * 2 + sde
                    g = h * NKVL + u
                    rows[1024 + 128 * tkk + 64 * sde + j] = \
                        NH * HD + g * HD + refdim
            W4 = wqkv[rows]
            wqk_a = np.ascontiguousarray(
                W4.reshape(10, 128, ND, 128).transpose(0, 3, 2, 1)).astype(bf)
            vrows = (NH + NKV) * HD + (h * NKVL * HD) + np.arange(NKVL * HD)
            Wv = wqkv[vrows]
            wv_a = np.ascontiguousarray(
                Wv.reshape(NKVL * HD, ND, 128).transpose(1, 2, 0)).astype(bf)
            worow = np.empty(1024, np.int64)
            dd = np.arange(HD)
            for t in range(8):
                a, bb = _pairing(t)
                for sde, ql in enumerate((a, bb)):
                    worow[128 * t + 64 * sde + dd] = (h * NQL + ql) * HD + dd
            woT = np.ascontiguousarray(wo[:, worow].T)
            wo_a = np.ascontiguousarray(
                woT.reshape(8, 128, 4, 512).transpose(2, 1, 0, 3)).astype(bf)
            in_maps.append({
                "xT": xTb,
                "wqk": wqk_a,
                "wv": wv_a,
                "wo_t": wo_a,
                "cos_d": cos_t,
                "sin_d": sin_t,
                "eye_d": eye,
                "msk_d": msk_arr,
            })
    return in_maps


def _run(x, freqs_cis, mask, wqkv, wo, trace=False):
    nc, mixed = _get_program(mask)
    in_maps = _host_inputs(x, freqs_cis, wqkv, wo, mixed)
    res = run_bass_kernel_spmd(nc, in_maps, list(range(N_CORES)), trace=trace)
    outs = [np.asarray(res.results[i]["outa"], np.float32).reshape(S, DIM) +
            np.asarray(res.results[i]["outb"], np.float32).reshape(S, DIM)
            for i in range(N_CORES)]
    full = np.stack([outs[2 * b] + outs[2 * b + 1] for b in range(B)])
    return full.astype(np.float32), res


def kernel(x, freqs_cis, mask, wqkv, wo):
    full, _ = _run(x, freqs_cis, mask, wqkv, wo, trace=False)
    return full


# revision 20
# speedup vs baseline: 1.4750x; 1.0178x over previous
"""Trainium2 Bass kernel for GQA attention (B=4, S=1024, DIM=2048, 32 Q heads /
8 KV heads, head_dim 64, rotary + causal mask, QKV + output projections).

Sharding: 8 cores = batch (4) x head-half (2). Each core computes one batch's
attention for 16 Q heads / 4 KV heads plus the partial output projection over
its 1024 y-features; the host sums the partials.

v3 schedule: bf16 operands (fp32 PSUM accumulation), fp16 partial outputs.
Projection processes both token halves per weight tile (weights DMA'd once).
Per-f iterations interleave proj(f+1) | scores(f) | AV(f-2) so the Act
engine's exp stream — the attention pacer — overlaps PE work; the causal
mask is added in PSUM via an identity-stationary bias matmul (no post-exp
vector masking).  The wo projection is split into two f-halves written to
separate DRAM partials (summed on host): the first half interleaves into the
attention tail, only the second trails.
"""

import hashlib
import sys

import numpy as np

for _p in ("/root/.axon_site/_ro/trn_rl_repo", "/opt/trn_rl_repo"):
    if _p not in sys.path:
        sys.path.append(_p)

import ml_dtypes
import concourse.bacc as bacc
import concourse.mybir as mybir
from concourse.tile import TileContext
from concourse.bass_utils import run_bass_kernel_spmd

F32 = mybir.dt.float32
BF16 = mybir.dt.bfloat16
F16 = mybir.dt.float16
AF = mybir.ActivationFunctionType

B, S, DIM = 4, 1024, 2048
NH, NKV, HD = 32, 8, 64
NQL, NKVL = 16, 4
N_CORES = 8
KT = S // 128
QPAIRS = 8
ND = DIM // 128
SCALE = 1.0 / 8.0


def _pairing(t):
    return (t, t + 4) if t < 4 else (t + 4, t + 8)


def _analyze_mask(M):
    """Block-classify the [S, S] bool mask (M[q, k]).

    Returns:
      runs[ki]  : list of (qs, qe, [(qt, mask_idx)]) maximal valid runs over q
      span[ki]  : (lo, hi) overall valid q range or None
      mixed     : list of unique mixed-block tiles in P^T layout [k, q]
    """
    runs, span = {}, {}
    mixed, midx = [], {}
    for ki in range(KT):
        rr, cur = [], None
        lo = hi = None
        for qt in range(KT):
            blk = M[128 * qt:128 * qt + 128, 128 * ki:128 * ki + 128]
            if (~blk).all():
                if cur is not None:
                    rr.append(tuple(cur))
                    cur = None
                continue
            mix = []
            if not blk.all():
                key = blk.tobytes()
                if key not in midx:
                    mixed.append(np.ascontiguousarray(blk.T).astype(np.float32))
                    midx[key] = len(mixed) - 1
                mix = [(qt, midx[key])]
            if cur is None:
                cur = [128 * qt, 128 * qt + 128, mix]
            else:
                cur[1] = 128 * qt + 128
                cur[2] += mix
            lo = 128 * qt if lo is None else lo
            hi = 128 * qt + 128
        if cur is not None:
            rr.append(tuple(cur))
        runs[ki] = rr
        span[ki] = (lo, hi) if lo is not None else None
    return runs, span, mixed


def _bank_subruns(runs_ki):
    """Split runs at 512 boundaries -> [(qs, qe, qc)], each inside one bank."""
    out = []
    for (qs, qe, _mix) in runs_ki:
        for qc in range(2):
            a, b = max(qs, 512 * qc), min(qe, 512 * qc + 512)
            if a < b:
                out.append((a, b, qc))
    return out


def _build_program(runs, span, n_mixed):
    nc = bacc.Bacc("TRN2", target_bir_lowering=False, debug=False,
                   num_devices=N_CORES)

    xT = nc.dram_tensor("xT", [DIM, S], BF16, kind="ExternalInput")
    wqk = nc.dram_tensor("wqk", [10, 128, ND, 128], BF16, kind="ExternalInput")
    wv = nc.dram_tensor("wv", [128, ND, NKVL * HD], BF16, kind="ExternalInput")
    wo_t = nc.dram_tensor("wo_t", [8, 128, 4, 512], BF16, kind="ExternalInput")
    cos_d = nc.dram_tensor("cos_d", [128, S], BF16, kind="ExternalInput")
    sin_d = nc.dram_tensor("sin_d", [128, S], BF16, kind="ExternalInput")
    eye_d = nc.dram_tensor("eye_d", [128, 128], BF16, kind="ExternalInput")
    nmx = max(n_mixed, 1)
    msk_d = nc.dram_tensor("msk_d", [nmx, 128, 128], BF16, kind="ExternalInput")
    outa_d = nc.dram_tensor("outa", [KT, 128, DIM], F16, kind="ExternalOutput")
    outb_d = nc.dram_tensor("outb", [KT, 128, DIM], F16, kind="ExternalOutput")

    with TileContext(nc) as tc:
      with tc.tile_pool(name="res", bufs=1) as res:
        qk_t = [res.tile([128, S], BF16, name=f"qk{t}", tag=f"qk{t}")
                for t in range(10)]
        v_t = [res.tile([128, NKVL * 65], BF16, name=f"v{k}", tag=f"v{k}")
               for k in range(KT)]
        y_t = [res.tile([128, S], BF16, name=f"y{t}", tag=f"y{t}")
               for t in range(QPAIRS)]
        cos_sb = res.tile([128, S], BF16, name="cos_sb")
        sin_sb = res.tile([128, S], BF16, name="sin_sb")
        eye_sb = res.tile([128, 128], BF16, name="eye_sb")
        msk_sb = [res.tile([128, 128], BF16, name=f"msk{i}", tag=f"msk{i}")
                  for i in range(n_mixed)]
        ones4 = res.tile([128, NKVL], BF16, name="ones4")
        woeh = [res.tile([128, 4, 512], BF16, name=f"woeh{i}", tag="woeh",
                         bufs=8) for i in range(8)]
        osb_t = [res.tile([128, 512], F16, name=f"osb{i}", tag="osb", bufs=6)
                 for i in range(64)]

        with (
            tc.tile_pool(name="st", bufs=1) as st,
            tc.tile_pool(name="pp", bufs=1, space="PSUM") as pp,
        ):
            # ---- DMA priority order: first-needed first ----
            wf = {}
            for f in [8, 9] + list(range(QPAIRS)):
                wf[f] = st.tile([128, ND, 128], BF16, name=f"wf{f}", tag="wf",
                                bufs=2)
            nc.sync.dma_start(wf[8][:], wqk[8])
            xd = {}
            for d in range(ND):
                xd[d] = st.tile([128, S], BF16, name=f"x_{d}", tag="xd",
                                bufs=16)
                nc.sync.dma_start(xd[d][:],
                                  xT[128 * d:128 * d + 128, :])
            nc.sync.dma_start(cos_sb[:], cos_d[:])
            nc.sync.dma_start(sin_sb[:], sin_d[:])
            nc.sync.dma_start(eye_sb[:], eye_d[:])
            for i in range(n_mixed):
                nc.sync.dma_start(msk_sb[i][:], msk_d[i])
            nc.gpsimd.memset(ones4[:], 1.0)
            wvt = st.tile([128, ND, NKVL * HD], BF16, name="wvt", tag="wvt",
                          bufs=1)
            nc.sync.dma_start(wvt[:], wv[:])

            def qkproj(f, dma=True):
                """Project feature tile f for both token halves + rope."""
                if dma:
                    nc.sync.dma_start(wf[f][:], wqk[f])
                for half in range(2):
                    tsl = slice(512 * half, 512 * half + 512)
                    ps = pp.tile([128, 512], F32, name=f"psqk{half}_{f}",
                                 tag="flex", bufs=2)
                    for d in range(ND):
                        nc.tensor.matmul(ps[:], wf[f][:, d, :],
                                         xd[d][:, tsl],
                                         start=(d == 0), stop=(d == ND - 1))
                    c0 = st.tile([128, 512], BF16, name=f"c0_{half}_{f}",
                                 tag="c0", bufs=2)
                    nc.scalar.copy(c0[:], ps[:])
                    sw = st.tile([128, 512], BF16, name=f"sw_{half}_{f}",
                                 tag="sw", bufs=2)
                    for blk in range(4):
                        sb = (blk ^ 1) * 32
                        nc.sync.dma_start(sw[32 * blk:32 * blk + 32, :],
                                          c0[sb:sb + 32, :])
                    t1 = st.tile([128, 512], BF16, name=f"t1_{half}_{f}",
                                 tag="t1", bufs=2)
                    t2 = st.tile([128, 512], BF16, name=f"t2_{half}_{f}",
                                 tag="t2", bufs=2)
                    nc.vector.tensor_mul(t1[:], c0[:], cos_sb[:, tsl])
                    nc.vector.tensor_mul(t2[:], sw[:], sin_sb[:, tsl])
                    nc.vector.tensor_add(qk_t[f][:, tsl], t1[:], t2[:])

            def vproj(half):
                for tq in range(4):
                    ki = 4 * half + tq
                    psv = pp.tile([128, NKVL * HD], F32, name=f"psv{ki}",
                                  tag="flex", bufs=2)
                    for d in range(ND):
                        nc.tensor.matmul(
                            psv[:],
                            xd[d][:, 128 * ki:128 * ki + 128],
                            wvt[:, d, :], start=(d == 0),
                            stop=(d == ND - 1))
                    vv = v_t[ki][:].rearrange("p (u c) -> p u c", u=NKVL, c=65)
                    nc.vector.tensor_copy(
                        vv[:, :, 0:64],
                        psv[:].rearrange("p (u c) -> p u c", u=NKVL, c=HD))
                    nc.gpsimd.tensor_copy(
                        vv[:, :, 64:65],
                        ones4[:].rearrange("p (u o) -> p u o", u=NKVL, o=1))

            def scores(p):
                """QK^T (+causal bias in PSUM) + exp for pair p."""
                tk = 0 if p < 4 else 1
                ptiles, poff = {}, {}
                for ki in range(KT):
                    if span[ki] is None:
                        continue
                    lo, hi = span[ki]
                    w = hi - lo
                    kwin = slice(128 * ki, 128 * ki + 128)
                    merge = w <= 512
                    if merge:
                        psS = pp.tile([128, 1024], F32, name=f"psS{p}{ki}",
                                      tag="psS", bufs=3)
                        pt = st.tile([128, 2 * w], BF16, name=f"P{p}_{ki}",
                                     tag=f"Pm_{ki}_{p % 3}", bufs=1)
                        for s in range(2):
                            ptiles[(s, ki)] = pt
                            poff[(s, ki)] = s * w - lo
                            ops = []
                            for (qs, qe, qc) in _bank_subruns(runs[ki]):
                                ops.append((
                                    psS[:, qs - lo + 512 * s:
                                        qe - lo + 512 * s],
                                    qk_t[8 + tk][64 * s:64 * s + 64, kwin],
                                    qk_t[p][64 * s:64 * s + 64, qs:qe]))
                            for (qs, qe, mix) in runs[ki]:
                                for (qt, mi) in mix:
                                    o = 128 * qt - lo + 512 * s
                                    ops.append((psS[:, o:o + 128], eye_sb[:],
                                                msk_sb[mi][:]))
                            for n, (o_, l_, r_) in enumerate(ops):
                                nc.tensor.matmul(
                                    o_, l_, r_, start=(n == 0),
                                    stop=(n == len(ops) - 1),
                                    skip_group_check=True)
                        psv2 = psS[:].rearrange("p (b c) -> p b c",
                                                b=2, c=512)[:, :, 0:w]
                        ptv = pt[:].rearrange("p (b c) -> p b c", b=2, c=w)
                        nc.scalar.activation(ptv, psv2, AF.Exp, scale=SCALE)
                        yield
                    else:
                        for s in range(2):
                            psS = pp.tile([128, 1024], F32,
                                          name=f"psS{p}{ki}{s}",
                                          tag="psS", bufs=3)
                            pt = st.tile([128, w], BF16, name=f"P{p}_{s}_{ki}",
                                         tag=f"P{s}_{ki}_{p % 3}", bufs=1)
                            ptiles[(s, ki)] = pt
                            poff[(s, ki)] = -lo
                            bank_first = {}
                            ops = []
                            for (qs, qe, qc) in _bank_subruns(runs[ki]):
                                st_flag = bank_first.setdefault(qc, True)
                                bank_first[qc] = False
                                ops.append((st_flag, psS[:, qs:qe],
                                            qk_t[8 + tk][64 * s:64 * s + 64,
                                                         kwin],
                                            qk_t[p][64 * s:64 * s + 64,
                                                    qs:qe]))
                            for (qs, qe, mix) in runs[ki]:
                                for (qt, mi) in mix:
                                    o = 128 * qt
                                    ops.append((False, psS[:, o:o + 128],
                                                eye_sb[:], msk_sb[mi][:]))
                            for n, (sf, o_, l_, r_) in enumerate(ops):
                                nc.tensor.matmul(
                                    o_, l_, r_, start=sf,
                                    stop=(n == len(ops) - 1),
                                    skip_group_check=True)
                            nc.scalar.activation(pt[:], psS[:, lo:hi],
                                                 AF.Exp, scale=SCALE)
                            yield
                # stash for av()
                ptp[p] = (ptiles, poff)

            def av(p):
                """AV + softmax-normalize into y_t[p] for pair p."""
                ptiles, poff = ptp[p]
                heads = _pairing(p)
                for s in range(2):
                    u = heads[s] // 4
                    for qc in range(2):
                        subs = []
                        for ki in range(KT):
                            if span[ki] is None:
                                continue
                            for (qs, qe, qq) in _bank_subruns(runs[ki]):
                                if qq == qc:
                                    subs.append((ki, qs, qe))
                        if not subs:
                            continue
                        psyf = pp.tile([128, 512], F32, name=f"psy{p}{s}{qc}",
                                       tag="flex", bufs=2)
                        psy = psyf[0:65, :]
                        for n, (ki, qs, qe) in enumerate(subs):
                            off = poff[(s, ki)]
                            nc.tensor.matmul(
                                psy[:, qs - 512 * qc:qe - 512 * qc],
                                v_t[ki][:, 65 * u:65 * u + 65],
                                ptiles[(s, ki)][:, qs + off:qe + off],
                                start=(n == 0), stop=(n == len(subs) - 1),
                                skip_group_check=True)
                        s_sb = st.tile([1, 512], F32, name=f"s{p}{s}{qc}",
                                       tag="ssb", bufs=2)
                        nc.vector.tensor_copy(s_sb[:], psy[64:65, :])
                        rf = st.tile([1, 512], F32, name=f"rf{p}{s}{qc}",
                                     tag="srf", bufs=2)
                        nc.vector.reciprocal_approx_fast(rf[:], s_sb[:])
                        rb = st.tile([64, 512], F32, name=f"rb{p}{s}{qc}",
                                     tag="rb", bufs=2)
                        nc.gpsimd.partition_broadcast(rb[:], rf[:])
                        nc.vector.tensor_mul(
                            y_t[p][64 * s:64 * s + 64,
                                   512 * qc:512 * qc + 512],
                            psy[0:64, :], rb[:])
                        yield

            def wo_wave(wave, fh):
                """One wo wave: 2 psum groups over f-half fh (4 f's)."""
                ec, th = wave // 4, wave % 4
                tts = (2 * th, 2 * th + 1)
                psos = {}
                for tt in tts:
                    psos[tt] = pp.tile([128, 512], F32,
                                       name=f"pso{fh}{ec}{tt}", tag="flex",
                                       bufs=2)
                for f in range(4 * fh, 4 * fh + 4):
                    wt = woeh[2 * ec + f // 4]
                    for tt in tts:
                        nc.tensor.matmul(
                            psos[tt][:],
                            y_t[f][:, 128 * tt:128 * tt + 128],
                            wt[:, f % 4, :],
                            start=(f % 4 == 0), stop=(f % 4 == 3))
                out_dram = outa_d if fh == 0 else outb_d
                for tt in tts:
                    osb = osb_t[32 * fh + 8 * ec + tt]
                    if fh == 0:
                        nc.vector.tensor_copy(osb[:], psos[tt][:])
                    else:
                        nc.scalar.copy(osb[:], psos[tt][:])
                    nc.sync.dma_start(
                        out_dram[tt, :, 512 * ec:512 * ec + 512], osb[:])

            def drain(gen):
                if gen is not None:
                    for _ in gen:
                        pass

            def step(gen, n=1):
                if gen is None:
                    return
                for _ in range(n):
                    try:
                        next(gen)
                    except StopIteration:
                        break

            ptp = {}
            sgen = {}
            agen = {}
            with nc.named_scope("main"):
                qkproj(8, dma=False)
                qkproj(9)
                vproj(0)
                qkproj(0)
                vproj(1)
                qkproj(1)
                # iterations f=2..7: proj(f) | scores(f-2) | AV(f-4)
                for f in range(2, 8):
                    if f == 6:
                        # prefetch wo weights (needed from the attn tail on)
                        for i in range(8):
                            nc.sync.dma_start(woeh[i][:], wo_t[i])
                    p = f - 2
                    sgen[p] = scores(p)
                    qkproj(f)
                    step(sgen[p], 6)
                    agen[p - 2] = av(p - 2) if p >= 2 else None
                    step(agen.get(p - 2), 2)
                    drain(sgen[p])
                    step(agen.get(p - 2), 2)
                    drain(agen.get(p - 2))
                # tail: scores(6,7), AV(4..7), wo-A waves interleaved
                sgen[6] = scores(6)
                drain(sgen[6])
                agen[4] = av(4)
                drain(agen[4])
                sgen[7] = scores(7)
                drain(sgen[7])
                wave_i = 0
                for p5 in (5, 6, 7):
                    agen[p5] = av(p5)
                    for _ in range(4):
                        step(agen[p5], 1)
                        wo_wave(wave_i, 0)
                        wave_i += 1
                for wave in range(wave_i, 16):
                    wo_wave(wave, 0)
                for wave in range(16):
                    wo_wave(wave, 1)

    nc.compile()
    return nc


_CACHE = {}


def _get_program(mask):
    M = np.asarray(mask).reshape(S, S).astype(bool)
    key = hashlib.md5(M.tobytes()).hexdigest()
    if key not in _CACHE:
        runs, span, mixed = _analyze_mask(M)
        nc = _build_program(runs, span, len(mixed))
        _CACHE[key] = (nc, mixed)
    return _CACHE[key]


def _host_inputs(x, freqs_cis, wqkv, wo, mixed):
    """Build the 8 per-core input maps."""
    bf = ml_dtypes.bfloat16
    x = np.asarray(x, dtype=np.float32)
    fc = np.asarray(freqs_cis, dtype=np.float32)
    wqkv = np.asarray(wqkv, dtype=np.float32)
    wo = np.asarray(wo, dtype=np.float32)

    cosv = fc[:, :, 0].T
    sinv = fc[:, :, 1].T
    cos_t = np.ascontiguousarray(np.tile(cosv, (4, 1))).astype(bf)
    sin_t = np.tile(sinv, (4, 1))
    sgn = np.ones((128, 1), np.float32)
    sgn[np.arange(128) % 64 < 32] = -1.0
    sin_t = np.ascontiguousarray(sin_t * sgn).astype(bf)

    nmx = max(len(mixed), 1)
    # mask shipped as additive bias tiles: 0 where valid, -30000 where masked
    msk_arr = np.zeros((nmx, 128, 128), bf)
    for i, m in enumerate(mixed):
        msk_arr[i] = ((m - 1.0) * 30000.0).astype(bf)
    eye = np.eye(128, dtype=bf)

    j = np.arange(HD)
    refdim = 2 * (j % 32) + (j // 32)

    in_maps = []
    for b in range(B):
        xTb = np.ascontiguousarray(x[b].T).astype(bf)
        for h in range(2):
            rows = np.empty(1280, np.int64)
            for t in range(8):
                a, bb = _pairing(t)
                for sde, ql in enumerate((a, bb)):
                    g = h * NQL + ql
                    rows[128 * t + 64 * sde + j] = g * HD + refdim
            for tkk in range(2):
                for sde in range(2):
                    u = tkk * 2 + sde
                    g = h * NKVL + u
                    rows[1024 + 128 * tkk + 64 * sde + j] = \
                        NH * HD + g * HD + refdim
            W4 = wqkv[rows]
            wqk_a = np.ascontiguousarray(
                W4.reshape(10, 128, ND, 128).transpose(0, 3, 2, 1)).astype(bf)
            vrows = (NH + NKV) * HD + (h * NKVL * HD) + np.arange(NKVL * HD)
            Wv = wqkv[vrows]
            wv_a = np.ascontiguousarray(
                Wv.reshape(NKVL * HD, ND, 128).transpose(2, 1, 0)).astype(bf)
            worow = np.empty(1024, np.int64)
            dd = np.arange(HD)
            for t in range(8):
                a, bb = _pairing(t)
                for sde, ql in enumerate((a, bb)):
                    worow[128 * t + 64 * sde + dd] = (h * NQL + ql) * HD + dd
            woT = np.ascontiguousarray(wo[:, worow].T)
            wo_a = np.ascontiguousarray(
                woT.reshape(8, 128, 4, 512).transpose(2, 0, 1, 3)
                .reshape(4, 2, 4, 128, 512).transpose(0, 1, 3, 2, 4)
                .reshape(8, 128, 4, 512)).astype(bf)
            in_maps.append({
                "xT": xTb,
                "wqk": wqk_a,
                "wv": wv_a,
                "wo_t": wo_a,
                "cos_d": cos_t,
                "sin_d": sin_t,
                "eye_d": eye,
                "msk_d": msk_arr,
            })
    return in_maps


def _run(x, freqs_cis, mask, wqkv, wo, trace=False):
    nc, mixed = _get_program(mask)
    in_maps = _host_inputs(x, freqs_cis, wqkv, wo, mixed)
    res = run_bass_kernel_spmd(nc, in_maps, list(range(N_CORES)), trace=trace)
    outs = [np.asarray(res.results[i]["outa"], np.float32).reshape(S, DIM) +
            np.asarray(res.results[i]["outb"], np.float32).reshape(S, DIM)
            for i in range(N_CORES)]
    full = np.stack([outs[2 * b] + outs[2 * b + 1] for b in range(B)])
    return full.astype(np.float32), res


def kernel(x, freqs_cis, mask, wqkv, wo):
    full, _ = _run(x, freqs_cis, mask, wqkv, wo, trace=False)
    return full
